# revision 1
# baseline (speedup 1.0000x reference)
"""Grok1-style GQA attention (S=2048, H=6144, 48 Q heads / 8 KV heads, rope,
softcap-30, causal) as a Bass/Tile kernel sharded over 8 NeuronCores.

Sharding: tensor-parallel across heads. Core c owns Q heads 6c..6c+5 and KV
head c. Each core computes its qkv projection slice, rope, causal softcap
attention for its 6 Q heads against its single KV head, and a partial
o_proj (its 768 columns of w_o). The host sums the 8 partial outputs.

Key numerics trick: softcap bounds scores to [-30, 30], so softmax is
computed as exp(30*tanh(s/30) - 30) with a *constant* bias — no running max.

Layouts (host-prepped, all transposed so the contraction dim is on SBUF
partitions):
  ht   [4,48,128,512] bf16  : ht[sc,hb,p,c] = hidden[sc*512+c, hb*128+p]
  wq   [8,128,48,128] bf16  : wq[ob,p,hb,o] = w_qkv_core[ob*128+o, hb*128+p]
  wo   [12,128,6,512] bf16  : wo[mc,p,fb,m] = (w_o[:,core]*MULT).T[fb*128+p, mc*512+m]
  cosf/sinf [128,2048] f32  : duplicated/sign-flipped rope tables (neox)
  triu [128,128] bf16       : triu[k,q] = 1 if q >= k else 0
"""

import sys, os
import numpy as np

sys.path.insert(0, "/opt/trn_rl_repo")

import ml_dtypes

import concourse.bass as bass
import concourse.mybir as mybir
import concourse.tile as tile
from concourse import bacc
from concourse.bass_utils import run_bass_kernel_spmd

F32 = mybir.dt.float32
F32R = mybir.dt.float32r
BF16 = mybir.dt.bfloat16
AF = mybir.ActivationFunctionType

S = 2048
HID = 6144
D = 128
NQ = 6          # q heads per core
N_CORES = 8
SCALE = D ** -0.5
SOFTCAP = 30.0
ATTN_MULT = 0.08838834764831845
ROPE_THETA = 10000.0

N_SC = 4        # s-chunks of 512
SCW = 512
N_HB = 48       # hidden 128-blocks
N_OB = 8        # output 128-blocks per core (6 Q | 1 K | 1 V)
N_MC = 12       # o_proj 512-col chunks
N_SB = 16       # s 128-blocks
N_FB = 6        # per-core o_proj feature 128-blocks (768/128)


def build_nc():
    nc = bacc.Bacc("TRN2", target_bir_lowering=False, debug=False, num_devices=N_CORES)

    ht_d = nc.dram_tensor("ht", [N_SC, N_HB, 128, SCW], BF16, kind="ExternalInput").ap()
    wq_d = nc.dram_tensor("wq", [N_OB, 128, N_HB, 128], BF16, kind="ExternalInput").ap()
    wo_d = nc.dram_tensor("wo", [N_MC, 128, N_FB, SCW], BF16, kind="ExternalInput").ap()
    cosf_d = nc.dram_tensor("cosf", [128, S], F32, kind="ExternalInput").ap()
    sinf_d = nc.dram_tensor("sinf", [128, S], F32, kind="ExternalInput").ap()
    triu_d = nc.dram_tensor("triu", [128, 128], BF16, kind="ExternalInput").ap()
    ones_col_d = nc.dram_tensor("ones_col", [128, 1], BF16, kind="ExternalInput").ap()
    ones_row_d = nc.dram_tensor("ones_row", [1, 128], F32, kind="ExternalInput").ap()
    ident_d = nc.dram_tensor("ident", [128, 128], BF16, kind="ExternalInput").ap()
    negcap_d = nc.dram_tensor("negcap", [128, 1], F32, kind="ExternalInput").ap()
    out_d = nc.dram_tensor("out", [S, HID], F32, kind="ExternalOutput").ap()

    from contextlib import ExitStack
    with tile.TileContext(nc) as tc, ExitStack() as ctx:
        const = ctx.enter_context(tc.tile_pool(name="const", bufs=1))
        pers = ctx.enter_context(tc.tile_pool(name="pers", bufs=1))
        htp = ctx.enter_context(tc.tile_pool(name="htp", bufs=49))
        wqp = ctx.enter_context(tc.tile_pool(name="wqp", bufs=2))
        ropep = ctx.enter_context(tc.tile_pool(name="ropep", bufs=2))
        tpool = ctx.enter_context(tc.tile_pool(name="tpool", bufs=3))
        ppool = ctx.enter_context(tc.tile_pool(name="ppool", bufs=4))
        rpool = ctx.enter_context(tc.tile_pool(name="rpool", bufs=2))
        bpool = ctx.enter_context(tc.tile_pool(name="bpool", bufs=2))
        wop = ctx.enter_context(tc.tile_pool(name="wop", bufs=2))
        outp = ctx.enter_context(tc.tile_pool(name="outp", bufs=3))
        ps_a = ctx.enter_context(tc.tile_pool(name="ps_a", bufs=2, space=bass.MemorySpace.PSUM))
        ps_s = ctx.enter_context(tc.tile_pool(name="ps_s", bufs=2, space=bass.MemorySpace.PSUM))
        ps_pv = ctx.enter_context(tc.tile_pool(name="ps_pv", bufs=2, space=bass.MemorySpace.PSUM))
        ps_o = ctx.enter_context(tc.tile_pool(name="ps_o", bufs=1, space=bass.MemorySpace.PSUM))
        ps_b = ctx.enter_context(tc.tile_pool(name="ps_b", bufs=1, space=bass.MemorySpace.PSUM))

        if True:
            cosf = const.tile([128, S], F32, tag="cosf", name="cosf")
            sinf = const.tile([128, S], F32, tag="sinf", name="sinf")
            triu = const.tile([128, 128], BF16, tag="triu", name="triu")
            ones_col = const.tile([128, 1], BF16, tag="ones_col", name="ones_col")
            ones_row = const.tile([1, 128], F32, tag="ones_row", name="ones_row")
            ident = const.tile([128, 128], BF16, tag="ident", name="ident")
            negcap = const.tile([128, 1], F32, tag="negcap", name="negcap")
            nc.sync.dma_start(cosf[:], cosf_d[:])
            nc.sync.dma_start(sinf[:], sinf_d[:])
            nc.sync.dma_start(triu[:], triu_d[:])
            nc.sync.dma_start(ones_col[:], ones_col_d[:])
            nc.sync.dma_start(ones_row[:], ones_row_d[:])
            nc.sync.dma_start(ident[:], ident_d[:])
            nc.sync.dma_start(negcap[:], negcap_d[:])

            QT = [pers.tile([128, S], BF16, tag=f"qt{h}", name=f"qt{h}") for h in range(NQ)]
            KT = pers.tile([128, S], BF16, tag="kt", name="kt")
            VT = pers.tile([128, S], BF16, tag="vt", name="vt")
            VN = pers.tile([128, S], BF16, tag="vn", name="vn")
            AOT = [pers.tile([128, S], BF16, tag=f"aot{h}", name=f"aot{h}") for h in range(NQ)]

            for sc in range(N_SC):
                scs = slice(sc * SCW, (sc + 1) * SCW)
                # ---- qkv projection for this s-chunk ----
                ht_tiles = []
                for hb in range(N_HB):
                    t = htp.tile([128, SCW], BF16, tag="ht", name="ht")
                    nc.sync.dma_start(t[:], ht_d[sc, hb])
                    ht_tiles.append(t)
                for ob in range(N_OB):
                    w_sb = wqp.tile([128, N_HB * 128], BF16, tag="wq", name="wq")
                    for qd in range(4):
                        nc.gpsimd.dma_start(
                            w_sb[:, qd * 12 * 128:(qd + 1) * 12 * 128],
                            wq_d[ob, :, qd * 12:(qd + 1) * 12])
                    ps = ps_a.tile([128, SCW], F32, tag="qkv", name="qkv")
                    for hb in range(N_HB):
                        nc.tensor.matmul(
                            ps[:],
                            lhsT=w_sb[:, hb * 128:(hb + 1) * 128],
                            rhs=ht_tiles[hb][:],
                            start=(hb == 0),
                            stop=(hb == N_HB - 1),
                        )
                    if ob <= NQ:  # rope for Q heads and K
                        rot = ropep.tile([128, SCW], F32, tag="rot", name="rot")
                        nc.scalar.copy(rot[0:64, :], ps[64:128, :])
                        nc.scalar.copy(rot[64:128, :], ps[0:64, :])
                        t1 = ropep.tile([128, SCW], F32, tag="t1", name="t1")
                        nc.vector.tensor_mul(t1[:], ps[:], cosf[:, scs])
                        nc.vector.tensor_mul(rot[:], rot[:], sinf[:, scs])
                        dst = QT[ob] if ob < NQ else KT
                        nc.vector.tensor_add(dst[:, scs], t1[:], rot[:])
                    else:
                        nc.vector.tensor_copy(VT[:, scs], ps[:])
                # ---- V transpose (natural [k, d] blocks) for this s-chunk ----
                for j in range(4):
                    kb = 4 * sc + j
                    tps = ps_a.tile([128, 128], BF16, tag="qkv", name="qkv")
                    nc.tensor.transpose(tps[:], VT[:, kb * 128:(kb + 1) * 128], ident[:])
                    nc.vector.tensor_copy(VN[:, kb * 128:(kb + 1) * 128], tps[:])
                # ---- attention for q-chunk qc = sc, all heads ----
                qc = sc
                nkb = 4 * qc + 4
                for h in range(NQ):
                    pv = ps_pv.tile([128, SCW], F32, tag="pv", name="pv")
                    oa = ps_o.tile([1, SCW], F32, tag="oa", name="oa")
                    for kb in range(nkb):
                        qs = max(qc * SCW, kb * 128)
                        off = qs - qc * SCW
                        w = SCW - off
                        sp = ps_s.tile([128, SCW], F32, tag="s", name="s")
                        nc.tensor.matmul(
                            sp[:, :w],
                            lhsT=KT[:, kb * 128:(kb + 1) * 128],
                            rhs=QT[h][:, qs:(qc + 1) * SCW],
                            start=True, stop=True,
                        )
                        tt = tpool.tile([128, SCW], F32, tag="t", name="t")
                        nc.scalar.activation(tt[:, :w], sp[:, :w], AF.Tanh,
                                             scale=SCALE / SOFTCAP)
                        pt = ppool.tile([128, SCW], BF16, tag="p", name="p")
                        nc.scalar.activation(pt[:, :w], tt[:, :w], AF.Exp,
                                             scale=SOFTCAP, bias=negcap[:])
                        if kb >= 4 * qc:
                            nc.vector.tensor_mul(pt[:, 0:128], pt[:, 0:128], triu[:])
                        nc.tensor.matmul(
                            pv[:, off:SCW],
                            lhsT=VN[:, kb * 128:(kb + 1) * 128],
                            rhs=pt[:, :w],
                            start=(kb == 0), stop=(kb == nkb - 1),
                        )
                        nc.tensor.matmul(
                            oa[0:1, off:SCW],
                            lhsT=ones_col[:],
                            rhs=pt[:, :w],
                            start=(kb == 0), stop=(kb == nkb - 1),
                        )
                    rr = rpool.tile([1, SCW], F32, tag="r", name="r")
                    nc.vector.reciprocal(rr[:], oa[:])
                    bp = ps_b.tile([128, SCW], F32, tag="b", name="b")
                    nc.tensor.matmul(bp[:], lhsT=ones_row[:],
                                     rhs=rr[:], start=True, stop=True)
                    bs = bpool.tile([128, SCW], F32, tag="bs", name="bs")
                    nc.scalar.copy(bs[:], bp[:])
                    nc.vector.tensor_mul(AOT[h][:, qc * SCW:(qc + 1) * SCW],
                                         pv[:], bs[:])

            # ---- o_proj partial: out[s, m] = sum_f aot[f, s] * woT[f, m] ----
            for mc in range(N_MC):
                wos = wop.tile([128, N_FB * SCW], BF16, tag="wo", name="wo")
                nc.sync.dma_start(wos[:], wo_d[mc])
                for sb in range(N_SB):
                    op = ps_a.tile([128, SCW], F32, tag="qkv", name="qkv")
                    for fb in range(N_FB):
                        nc.tensor.matmul(
                            op[:],
                            lhsT=AOT[fb][:, sb * 128:(sb + 1) * 128],
                            rhs=wos[:, fb * SCW:(fb + 1) * SCW],
                            start=(fb == 0), stop=(fb == N_FB - 1),
                        )
                    ot = outp.tile([128, SCW], F32, tag="out", name="out")
                    nc.vector.tensor_copy(ot[:], op[:])
                    nc.sync.dma_start(
                        out_d[sb * 128:(sb + 1) * 128, mc * SCW:(mc + 1) * SCW], ot[:])

    nc.compile()
    return nc


def prep_inputs(positions, hidden_states, w_qkv, w_o):
    """Host-side shard + relayout. Returns per-core input maps."""
    bf = ml_dtypes.bfloat16
    pos = np.asarray(positions).astype(np.float32)
    hidden = np.ascontiguousarray(np.asarray(hidden_states, dtype=np.float32))
    w_qkv = np.asarray(w_qkv, dtype=np.float32)
    w_o = np.asarray(w_o, dtype=np.float32)

    # rope tables (neox): freqs [S, 64]
    inv_freq = 1.0 / (ROPE_THETA ** (np.arange(0, D, 2, dtype=np.float32) / D))
    freqs = pos[:, None] * inv_freq[None, :]
    cos = np.cos(freqs).T.astype(np.float32)   # [64, S]
    sin = np.sin(freqs).T.astype(np.float32)
    cosf = np.concatenate([cos, cos], axis=0)               # [128, S]
    sinf = np.concatenate([-sin, sin], axis=0)

    triu = np.triu(np.ones((128, 128), np.float32)).astype(bf)  # [k, q]: q >= k
    ones_col = np.ones((128, 1), np.float32).astype(bf)
    ones_row = np.ones((1, 128), np.float32)
    ident = np.eye(128, dtype=np.float32).astype(bf)

    # ht[sc, hb, p, c] = hidden[sc*512+c, hb*128+p]
    ht = np.ascontiguousarray(
        hidden.reshape(N_SC, SCW, N_HB, 128).transpose(0, 2, 3, 1)).astype(bf)

    in_maps = []
    for c in range(N_CORES):
        q_rows = w_qkv[c * NQ * D:(c + 1) * NQ * D]          # [768, 6144]
        k_rows = w_qkv[HID + c * D:HID + (c + 1) * D]        # [128, 6144]
        v_rows = w_qkv[HID + 8 * D + c * D:HID + 8 * D + (c + 1) * D]
        wq_c = np.concatenate([q_rows, k_rows, v_rows], axis=0)  # [1024, 6144]
        # wq[ob, p, hb, o] = wq_c[ob*128+o, hb*128+p]
        wq_arr = np.ascontiguousarray(
            wq_c.reshape(N_OB, 128, N_HB, 128).transpose(0, 3, 2, 1)).astype(bf)
        wo_c = (w_o[:, c * NQ * D:(c + 1) * NQ * D] * ATTN_MULT).T  # [768, 6144]
        # wo[mc, p, fb, m] = wo_c[fb*128+p, mc*512+m]
        wo_arr = np.ascontiguousarray(
            wo_c.reshape(N_FB, 128, N_MC, SCW).transpose(2, 1, 0, 3)).astype(bf)
        in_maps.append({
            "ht": ht, "wq": wq_arr, "wo": wo_arr,
            "cosf": cosf, "sinf": sinf, "triu": triu,
            "ones_col": ones_col, "ones_row": ones_row, "ident": ident,
            "negcap": np.full((128, 1), -SOFTCAP, np.float32),
        })
    return in_maps


_NC_CACHE = None


def _get_nc():
    global _NC_CACHE
    if _NC_CACHE is None:
        _NC_CACHE = build_nc()
    return _NC_CACHE


def kernel(positions, hidden_states, w_qkv, w_o, _trace=False, _trace_kwargs=None):
    nc = _get_nc()
    in_maps = prep_inputs(positions, hidden_states, w_qkv, w_o)
    res = run_bass_kernel_spmd(nc, in_maps, list(range(N_CORES)),
                               trace=_trace, **(_trace_kwargs or {}))
    out = np.zeros((S, HID), np.float32)
    for c in range(N_CORES):
        out += res.results[c]["out"]
    out = out.astype(np.asarray(hidden_states).dtype)
    kernel.last_results = res
    return out



# revision 6
# speedup vs baseline: 1.3003x; 1.3003x over previous
"""Grok1-style GQA attention (S=2048, H=6144, 48 Q heads / 8 KV heads, rope,
softcap-30, causal) as a Bass/Tile kernel sharded over 8 NeuronCores.

Sharding: tensor-parallel across heads. Core c owns Q heads 6c..6c+5 and KV
head c. Each core computes its qkv projection slice, rope, causal softcap
attention for its 6 Q heads against its single KV head, and a partial
o_proj (its 768 columns of w_o). The host sums the 8 partial outputs.

Numerics: softcap bounds scores to [-30, 30], so softmax is computed as
exp(30*tanh(s/30) - 30) with a *constant* bias — no running max.

v2 design (vs the naive per-block version):
 - The softmax denominator comes free from the PV matmul: V is augmented
   with a ones column (VN blocks are [k,129], col 128 = 1), and PV is done
   in [q, d] orientation (lhsT = probs [k,q], rhs = V_aug [k,129]) so the
   per-query denominator lands on the q PARTITION axis -> cheap per-partition
   reciprocal + scale on the vector engine. This kills the M=1 row-sum
   matmuls, the K=1 broadcast matmuls and the single-lane reciprocals.
 - tanh/exp run on [128, <=1024] batches spanning 2 PSUM banks (fewer ACT
   instructions, less fixed overhead).
 - Software pipelining by emission order: attention of chunk i is
   interleaved with the QKV projection of chunk i+1 (and attention of the
   last chunk with the first 3/4 of o_proj) so the tensor engine never
   stalls on the scalar engine and HAM stays warm.
 - Normalized attention outputs are transposed back to [d, q] in bulk at
   chunk end (PE transpose + DVE copy) for the o_proj lhsT.

Layouts (host-prepped, contraction dim on SBUF partitions):
  ht   [4,48,128,512] bf16  : ht[sc,hb,p,c] = hidden[sc*512+c, hb*128+p]
  wq   [8,128,48,128] bf16  : wq[ob,p,hb,o] = w_qkv_core[ob*128+o, hb*128+p]
  wo   [12,128,6,512] bf16  : wo[mc,p,fb,m] = (w_o[:,core]*MULT).T[fb*128+p, mc*512+m]
  cosf/sinf [128,2048] f32  : duplicated/sign-flipped rope tables (neox)
  triu [128,128] bf16       : triu[k,q] = 1 if q >= k else 0
"""

import sys
import numpy as np
from collections import deque

sys.path.insert(0, "/opt/trn_rl_repo")

import ml_dtypes

import concourse.bass as bass
import concourse.mybir as mybir
import concourse.tile as tile
from concourse import bacc
from concourse.bass_utils import run_bass_kernel_spmd

F32 = mybir.dt.float32
BF16 = mybir.dt.bfloat16
AF = mybir.ActivationFunctionType

S = 2048
HID = 6144
D = 128
NQ = 6          # q heads per core
N_CORES = 8
SCALE = D ** -0.5
SOFTCAP = 30.0
ATTN_MULT = 0.08838834764831845
ROPE_THETA = 10000.0

N_SC = 4        # s-chunks of 512
SCW = 512
N_HB = 48       # hidden 128-blocks
N_OB = 8        # output 128-blocks per core (6 Q | 1 K | 1 V)
N_MC = 12       # o_proj 512-col chunks
N_SB = 16       # s 128-blocks
N_FB = 6        # per-core o_proj feature 128-blocks (768/128)


def build_nc():
    nc = bacc.Bacc("TRN2", target_bir_lowering=False, debug=False, num_devices=N_CORES)

    ht_d = nc.dram_tensor("ht", [N_SC, N_HB, 128, SCW], BF16, kind="ExternalInput").ap()
    wq_d = nc.dram_tensor("wq", [N_OB, 128, N_HB, 128], BF16, kind="ExternalInput").ap()
    wo_d = nc.dram_tensor("wo", [N_MC, 128, N_FB, SCW], BF16, kind="ExternalInput").ap()
    cosf_d = nc.dram_tensor("cosf", [128, S], F32, kind="ExternalInput").ap()
    sinf_d = nc.dram_tensor("sinf", [128, S], F32, kind="ExternalInput").ap()
    triu_d = nc.dram_tensor("triu", [128, 128], BF16, kind="ExternalInput").ap()
    ident_d = nc.dram_tensor("ident", [128, 128], BF16, kind="ExternalInput").ap()
    negcap_d = nc.dram_tensor("negcap", [128, 1], F32, kind="ExternalInput").ap()
    out_d = nc.dram_tensor("out", [S, HID], BF16, kind="ExternalOutput").ap()

    from contextlib import ExitStack
    with tile.TileContext(nc) as tc, ExitStack() as ctx:
        const = ctx.enter_context(tc.tile_pool(name="const", bufs=1))
        pers = ctx.enter_context(tc.tile_pool(name="pers", bufs=1))
        htp = ctx.enter_context(tc.tile_pool(name="htp", bufs=48))
        wqp = ctx.enter_context(tc.tile_pool(name="wqp", bufs=2))
        wop = ctx.enter_context(tc.tile_pool(name="wop", bufs=2))
        ropep = ctx.enter_context(tc.tile_pool(name="ropep", bufs=4))
        stp = ctx.enter_context(tc.tile_pool(name="stp", bufs=2))
        ptp = ctx.enter_context(tc.tile_pool(name="ptp", bufs=8))
        nsp = ctx.enter_context(tc.tile_pool(name="nsp", bufs=26))
        rp = ctx.enter_context(tc.tile_pool(name="rp", bufs=6))
        otp = ctx.enter_context(tc.tile_pool(name="otp", bufs=3))
        ps_a = ctx.enter_context(tc.tile_pool(name="ps_a", bufs=2, space=bass.MemorySpace.PSUM))
        ps_sc = ctx.enter_context(tc.tile_pool(name="ps_sc", bufs=2, space=bass.MemorySpace.PSUM))
        ps_pv = ctx.enter_context(tc.tile_pool(name="ps_pv", bufs=2, space=bass.MemorySpace.PSUM))

        # ---------- persistent SBUF tiles (per s-chunk for precise deps) ----
        QT = [[pers.tile([128, SCW], BF16, tag=f"qt{h}_{c}", name=f"qt{h}_{c}")
               for c in range(N_SC)] for h in range(NQ)]
        KT = [pers.tile([128, SCW], BF16, tag=f"kt{c}", name=f"kt{c}") for c in range(N_SC)]
        VT = [pers.tile([128, SCW], BF16, tag=f"vt{c}", name=f"vt{c}") for c in range(N_SC)]
        VN = [pers.tile([128, 4 * 129], BF16, tag=f"vn{c}", name=f"vn{c}") for c in range(N_SC)]
        AOT = [[pers.tile([128, SCW], BF16, tag=f"aot{h}_{c}", name=f"aot{h}_{c}")
                for c in range(N_SC)] for h in range(NQ)]

        ht_tiles = {}

        def emit_ht_dma(sc):
            lst = []
            for hb in range(N_HB):
                t = htp.tile([128, SCW], BF16, tag="ht", name="ht")
                nc.sync.dma_start(t[:], ht_d[sc, hb])
                lst.append(t)
            ht_tiles[sc] = lst

        # hidden chunk 0 first so the first matmul can start ASAP
        emit_ht_dma(0)

        cosf = const.tile([128, S], F32, tag="cosf", name="cosf")
        sinf = const.tile([128, S], F32, tag="sinf", name="sinf")
        triu = const.tile([128, 128], BF16, tag="triu", name="triu")
        ident = const.tile([128, 128], BF16, tag="ident", name="ident")
        negcap = const.tile([128, 1], F32, tag="negcap", name="negcap")
        nc.sync.dma_start(triu[:], triu_d[:])
        nc.sync.dma_start(ident[:], ident_d[:])
        nc.sync.dma_start(negcap[:], negcap_d[:])
        nc.sync.dma_start(cosf[:], cosf_d[:])
        nc.sync.dma_start(sinf[:], sinf_d[:])

        # ---------------- QKV projection units (2 per ob) -------------------
        def make_qkv_units(sc):
            state = {}
            scs = slice(sc * SCW, (sc + 1) * SCW)

            def unit_a(ob):
                w = wqp.tile([128, N_HB * 128], BF16, tag="wq", name="wq")
                ps = ps_a.tile([128, SCW], F32, tag="acc", name="acc")
                state[ob] = (w, ps)
                for qd in (0, 1):
                    nc.gpsimd.dma_start(
                        w[:, qd * 1536:(qd + 1) * 1536],
                        wq_d[ob, :, qd * 12:(qd + 1) * 12])
                for hb in range(24):
                    nc.tensor.matmul(
                        ps[:], lhsT=w[:, hb * 128:(hb + 1) * 128],
                        rhs=ht_tiles[sc][hb][:], start=(hb == 0), stop=False)

            def unit_b(ob):
                w, ps = state.pop(ob)
                for qd in (2, 3):
                    nc.gpsimd.dma_start(
                        w[:, qd * 1536:(qd + 1) * 1536],
                        wq_d[ob, :, qd * 12:(qd + 1) * 12])
                for hb in range(24, N_HB):
                    nc.tensor.matmul(
                        ps[:], lhsT=w[:, hb * 128:(hb + 1) * 128],
                        rhs=ht_tiles[sc][hb][:], start=False, stop=(hb == N_HB - 1))
                if ob == 7:
                    nc.vector.tensor_copy(VT[sc][:], ps[:])
                    nc.vector.memset(VN[sc][:], 1.0)
                    tr = ps_sc.tile([128, 1024], BF16, tag="sc", name="sc")
                    for j in range(4):
                        nc.tensor.transpose(
                            tr[:, j * 128:(j + 1) * 128],
                            VT[sc][:, j * 128:(j + 1) * 128], ident[:])
                    for j in range(4):
                        nc.vector.tensor_copy(
                            VN[sc][:, j * 129:j * 129 + 128],
                            tr[:, j * 128:(j + 1) * 128])
                else:
                    rot = ropep.tile([128, SCW], F32, tag="rot", name="rot")
                    t1 = ropep.tile([128, SCW], F32, tag="t1", name="t1")
                    nc.scalar.copy(rot[0:64, :], ps[64:128, :])
                    nc.scalar.copy(rot[64:128, :], ps[0:64, :])
                    nc.vector.tensor_mul(t1[:], ps[:], cosf[:, scs])
                    nc.vector.tensor_mul(rot[:], rot[:], sinf[:, scs])
                    dst = QT[ob][sc] if ob < NQ else KT[sc]
                    nc.vector.tensor_add(dst[:], t1[:], rot[:])

            units = []
            for ob in (6, 7, 0, 1, 2, 3, 4, 5):   # K, V first, then Q heads
                units.append(lambda ob=ob: unit_a(ob))
                units.append(lambda ob=ob: unit_b(ob))
            return units

        # ---------------- o_proj units --------------------------------------
        wo_state = {}

        def oproj_dma(mc, gen):
            def f():
                w = wop.tile([128, N_FB * SCW], BF16, tag="wo", name="wo")
                nc.sync.dma_start(w[:], wo_d[mc])
                wo_state[(mc, gen)] = w
            return f

        def oproj_mm(mc, sb, gen, eng):
            def f():
                w = wo_state[(mc, gen)]
                ps = ps_a.tile([128, SCW], F32, tag="acc", name="acc")
                for fb in range(N_FB):
                    nc.tensor.matmul(
                        ps[:],
                        lhsT=AOT[fb][sb // 4][:, (sb % 4) * 128:(sb % 4) * 128 + 128],
                        rhs=w[:, fb * SCW:(fb + 1) * SCW],
                        start=(fb == 0), stop=(fb == N_FB - 1))
                ot = otp.tile([128, SCW], BF16, tag="ot", name="ot")
                if eng == 0:
                    nc.vector.tensor_copy(ot[:], ps[:])
                else:
                    nc.scalar.copy(ot[:], ps[:])
                nc.sync.dma_start(
                    out_d[sb * 128:(sb + 1) * 128, mc * SCW:(mc + 1) * SCW], ot[:])
            return f

        def make_oproj_units(sb_list, gen, eng):
            units = [oproj_dma(0, gen), oproj_dma(1, gen)]
            for mc in range(N_MC):
                for i, sb in enumerate(sb_list):
                    units.append(oproj_mm(mc, sb, gen, eng))
                    if i == 0 and mc + 2 < N_MC:
                        units.append(oproj_dma(mc + 2, gen))
            return units

        # ---------------- filler machinery ----------------------------------
        filler = deque()

        def drain_sched(idx, slots):
            # evenly distribute whatever is currently queued over the
            # remaining drain slots of this chunk
            total = len(filler) + drain_sched.done
            take = (idx + 1) * total // slots - drain_sched.done
            for _ in range(max(0, take)):
                if not filler:
                    break
                filler.popleft()()
                drain_sched.done += 1
        drain_sched.done = 0

        # ---------------- attention -----------------------------------------
        def batches_for(qc):
            bs = []
            for i in range(2 * qc):
                bs.append(dict(blocks=[(2 * i, 0, 512, 0), (2 * i + 1, 512, 512, 0)],
                               width=1024, diag=[]))
            base = 4 * qc
            bs.append(dict(blocks=[(base, 0, 512, 0), (base + 1, 512, 384, 128)],
                           width=896, diag=[0, 1]))
            bs.append(dict(blocks=[(base + 2, 0, 256, 256), (base + 3, 256, 128, 384)],
                           width=384, diag=[0, 1]))
            return bs

        def emit_attn(qc):
            ns_all = []
            bs_proto = batches_for(qc)
            n_slots = NQ * (len(bs_proto) + 4)
            slot = 0
            drain_sched.done = 0
            for h in range(NQ):
                pt_map = {}
                for b in bs_proto:
                    sc_t = ps_sc.tile([128, 1024], F32, tag="sc", name="sc")
                    for (kb, off, w, q_lo) in b["blocks"]:
                        nc.tensor.matmul(
                            sc_t[:, off:off + w],
                            lhsT=KT[kb // 4][:, (kb % 4) * 128:(kb % 4) * 128 + 128],
                            rhs=QT[h][qc][:, q_lo:q_lo + w],
                            start=True, stop=True)
                    wdt = b["width"]
                    st = stp.tile([128, 1024], BF16, tag="st", name="st")
                    nc.scalar.activation(st[:, :wdt], sc_t[:, :wdt], AF.Tanh,
                                         scale=SCALE / SOFTCAP)
                    pt = ptp.tile([128, 1024], BF16, tag="pt", name="pt")
                    nc.scalar.activation(pt[:, :wdt], st[:, :wdt], AF.Exp,
                                         scale=SOFTCAP, bias=negcap[:])
                    for bi in b["diag"]:
                        (kb, off, w, q_lo) = b["blocks"][bi]
                        g = kb - 4 * qc
                        dcol = off + (g * 128 - q_lo)
                        nc.vector.tensor_mul(pt[:, dcol:dcol + 128],
                                             pt[:, dcol:dcol + 128], triu[:])
                    for (kb, off, w, q_lo) in b["blocks"]:
                        pt_map[kb] = (pt, off, q_lo)
                    drain_sched(slot, n_slots); slot += 1
                for j in range(4):
                    qb = 4 * qc + j
                    pv = ps_pv.tile([128, 129], F32, tag="pv", name="pv")
                    for kb in range(qb + 1):
                        pt, off, q_lo = pt_map[kb]
                        col = off + (j * 128 - q_lo)
                        nc.tensor.matmul(
                            pv[:],
                            lhsT=pt[:, col:col + 128],
                            rhs=VN[kb // 4][:, (kb % 4) * 129:(kb % 4) * 129 + 129],
                            start=(kb == 0), stop=(kb == qb))
                    r = rp.tile([128, 1], F32, tag="r", name="r")
                    nc.vector.reciprocal(r[:], pv[:, 128:129])
                    n = nsp.tile([128, 128], BF16, tag="ns", name="ns")
                    nc.vector.tensor_scalar_mul(n[:], pv[:, 0:128], r[:])
                    ns_all.append((h, j, n))
                    drain_sched(slot, n_slots); slot += 1
            # bulk transpose of normalized outputs back to [d, q]
            for g0 in range(0, len(ns_all), 8):
                grp = ns_all[g0:g0 + 8]
                tr = ps_sc.tile([128, 1024], BF16, tag="sc", name="sc")
                for s_i, (h, j, n) in enumerate(grp):
                    nc.tensor.transpose(tr[:, s_i * 128:(s_i + 1) * 128], n[:], ident[:])
                for s_i, (h, j, n) in enumerate(grp):
                    nc.vector.tensor_copy(AOT[h][qc][:, j * 128:(j + 1) * 128],
                                          tr[:, s_i * 128:(s_i + 1) * 128])

        # ================= emission =========================================
        for u in make_qkv_units(0):
            u()
        for qc in range(N_SC):
            if qc + 1 < N_SC:
                emit_ht_dma(qc + 1)
                filler.extend(make_qkv_units(qc + 1))
            else:
                filler.extend(make_oproj_units(list(range(12)), gen=0, eng=0))
            emit_attn(qc)
            while filler:
                filler.popleft()()
        for u in make_oproj_units([12, 13, 14, 15], gen=1, eng=1):
            u()

    nc.compile()
    return nc


def prep_inputs(positions, hidden_states, w_qkv, w_o):
    """Host-side shard + relayout. Returns per-core input maps."""
    bf = ml_dtypes.bfloat16
    pos = np.asarray(positions).astype(np.float32)
    hidden = np.ascontiguousarray(np.asarray(hidden_states, dtype=np.float32))
    w_qkv = np.asarray(w_qkv, dtype=np.float32)
    w_o = np.asarray(w_o, dtype=np.float32)

    # rope tables (neox): freqs [S, 64]
    inv_freq = 1.0 / (ROPE_THETA ** (np.arange(0, D, 2, dtype=np.float32) / D))
    freqs = pos[:, None] * inv_freq[None, :]
    cos = np.cos(freqs).T.astype(np.float32)   # [64, S]
    sin = np.sin(freqs).T.astype(np.float32)
    cosf = np.concatenate([cos, cos], axis=0)               # [128, S]
    sinf = np.concatenate([-sin, sin], axis=0)

    triu = np.triu(np.ones((128, 128), np.float32)).astype(bf)  # [k, q]: q >= k
    ident = np.eye(128, dtype=np.float32).astype(bf)

    # ht[sc, hb, p, c] = hidden[sc*512+c, hb*128+p]
    ht = np.ascontiguousarray(
        hidden.reshape(N_SC, SCW, N_HB, 128).transpose(0, 2, 3, 1)).astype(bf)

    in_maps = []
    for c in range(N_CORES):
        q_rows = w_qkv[c * NQ * D:(c + 1) * NQ * D]          # [768, 6144]
        k_rows = w_qkv[HID + c * D:HID + (c + 1) * D]        # [128, 6144]
        v_rows = w_qkv[HID + 8 * D + c * D:HID + 8 * D + (c + 1) * D]
        wq_c = np.concatenate([q_rows, k_rows, v_rows], axis=0)  # [1024, 6144]
        # wq[ob, p, hb, o] = wq_c[ob*128+o, hb*128+p]
        wq_arr = np.ascontiguousarray(
            wq_c.reshape(N_OB, 128, N_HB, 128).transpose(0, 3, 2, 1)).astype(bf)
        wo_c = (w_o[:, c * NQ * D:(c + 1) * NQ * D] * ATTN_MULT).T  # [768, 6144]
        # wo[mc, p, fb, m] = wo_c[fb*128+p, mc*512+m]
        wo_arr = np.ascontiguousarray(
            wo_c.reshape(N_FB, 128, N_MC, SCW).transpose(2, 1, 0, 3)).astype(bf)
        in_maps.append({
            "ht": ht, "wq": wq_arr, "wo": wo_arr,
            "cosf": cosf, "sinf": sinf, "triu": triu, "ident": ident,
            "negcap": np.full((128, 1), -SOFTCAP, np.float32),
        })
    return in_maps


_NC_CACHE = None


def _get_nc():
    global _NC_CACHE
    if _NC_CACHE is None:
        _NC_CACHE = build_nc()
    return _NC_CACHE


def kernel(positions, hidden_states, w_qkv, w_o, _trace=False, _trace_kwargs=None):
    nc = _get_nc()
    in_maps = prep_inputs(positions, hidden_states, w_qkv, w_o)
    res = run_bass_kernel_spmd(nc, in_maps, list(range(N_CORES)),
                               trace=_trace, **(_trace_kwargs or {}))
    out = np.zeros((S, HID), np.float32)
    for c in range(N_CORES):
        out += res.results[c]["out"].astype(np.float32)
    out = out.astype(np.asarray(hidden_states).dtype)
    kernel.last_results = res
    return out
